# revision 1
# baseline (speedup 1.0000x reference)
"""MoE gate routing kernel for Trainium2 (8 NeuronCores, SPMD token-parallel).

Problem: scores = sigmoid(x @ weight.T); s = scores + bias;
group top-2 sums -> top-4 groups mask -> global top-8 -> gather original
scores -> normalize * 2.5. Returns (w [T,8] f32, idx [T,8] int32).

Sharding: token dim split 8 ways; weight/bias replicated. Inside each core:
x^T tiles (prepared host-side) feed fp32 matmuls (exact top-k needs fp32);
routing block runs on DVE with max/max_index/match_replace ISA ops.
"""
import sys

if "/opt/trn_rl_repo" not in sys.path:
    sys.path.insert(0, "/opt/trn_rl_repo")

import numpy as np

T, D, E = 16384, 7168, 256
G, KG, KTOP = 8, 4, 8
ROUTE_SCALE = 2.5
NCORES = 8
TCORE = T // NCORES          # 2048 tokens per core
NT = TCORE // 128            # 16 token tiles per core
KD = D // 128                # 56 contraction chunks
BIG = 1e30

_CACHE = {}


def _build():
    import concourse.bacc as bacc
    import concourse.mybir as mybir
    import concourse.tile as tile
    from contextlib import ExitStack

    F32 = mybir.dt.float32
    U32 = mybir.dt.uint32
    X = mybir.AxisListType.X
    Alu = mybir.AluOpType

    nc = bacc.Bacc(None, target_bir_lowering=False, debug=False)

    xt_d = nc.dram_tensor("xt", [NT, 128, KD * 128], F32, kind="ExternalInput")
    wt_d = nc.dram_tensor("wt", [128, KD * E], F32, kind="ExternalInput")
    bi_d = nc.dram_tensor("bi", [128, 2 * E], F32, kind="ExternalInput")
    w_out_d = nc.dram_tensor("w_out", [128, NT * KTOP], F32, kind="ExternalOutput")
    idx_out_d = nc.dram_tensor("idx_out", [128, NT * KTOP], U32, kind="ExternalOutput")

    with tile.TileContext(nc) as tc, ExitStack() as ctx:
        const = ctx.enter_context(tc.tile_pool(name="const", bufs=1))
        outp = ctx.enter_context(tc.tile_pool(name="outp", bufs=1))
        xpool = ctx.enter_context(tc.tile_pool(name="xp", bufs=3))
        pspool = ctx.enter_context(tc.tile_pool(name="ps", bufs=2, space="PSUM"))
        work = ctx.enter_context(tc.tile_pool(name="work", bufs=2))
        small = ctx.enter_context(tc.tile_pool(name="small", bufs=2))

        wt_sb = const.tile([128, KD, E], F32)
        bi_sb = const.tile([128, 2 * E], F32)
        nc.sync.dma_start(wt_sb[:], wt_d[:])
        nc.sync.dma_start(bi_sb[:], bi_d[:])
        bias_sb = bi_sb[:, 0:E]
        iota_sb = bi_sb[:, E:2 * E]

        w_acc = outp.tile([128, NT, KTOP], F32)
        idx_acc = outp.tile([128, NT, KTOP], U32)

        for t in range(NT):
            xt = xpool.tile([128, KD, 128], F32, tag="xt")
            nc.sync.dma_start(xt[:], xt_d[t])

            ps = pspool.tile([128, E], F32, tag="ps")
            for k in range(KD):
                nc.tensor.matmul(
                    ps[:], xt[:, k, :], wt_sb[:, k, :],
                    start=(k == 0), stop=(k == KD - 1),
                )

            orig = work.tile([128, E], F32, tag="orig")
            nc.scalar.activation(orig[:], ps[:], mybir.ActivationFunctionType.Sigmoid)

            s = work.tile([128, E], F32, tag="s")
            nc.vector.tensor_add(s[:], orig[:], bias_sb)
            sg = s[:].rearrange("p (g f) -> p g f", g=G)

            m1 = small.tile([128, G], F32, tag="m1")
            nc.vector.reduce_max(m1[:], sg, axis=X)
            tmp = work.tile([128, E], F32, tag="tmp")
            nc.vector.match_replace(
                out=tmp[:], in_to_replace=m1[:], in_values=s[:], imm_value=-BIG
            )
            m2 = small.tile([128, G], F32, tag="m2")
            nc.vector.reduce_max(m2[:], tmp[:].rearrange("p (g f) -> p g f", g=G), axis=X)
            gs = small.tile([128, G], F32, tag="gs")
            nc.vector.tensor_add(gs[:], m1[:], m2[:])

            g8 = small.tile([128, 8], F32, tag="g8")
            nc.vector.max(out=g8[:], in_=gs[:])
            pen = small.tile([128, G], F32, tag="pen")
            nc.vector.tensor_scalar(
                pen[:], gs[:], g8[:, 3:4], -BIG, op0=Alu.is_lt, op1=Alu.mult
            )

            masked = work.tile([128, E], F32, tag="masked")
            pen_b = pen[:].unsqueeze(2).broadcast_to([128, G, E // G])
            nc.vector.tensor_tensor(
                out=masked[:].rearrange("p (g f) -> p g f", g=G),
                in0=sg, in1=pen_b, op=Alu.add,
            )

            v8 = small.tile([128, KTOP], F32, tag="v8")
            nc.vector.max(out=v8[:], in_=masked[:])
            nc.vector.max_index(idx_acc[:, t, :], v8[:], masked[:])

            idxf = small.tile([128, KTOP], F32, tag="idxf")
            nc.vector.tensor_copy(idxf[:], idx_acc[:, t, :])
            w8raw = small.tile([128, KTOP], F32, tag="w8raw")
            scratch = work.tile([128, E], F32, tag="scratch")
            for j in range(KTOP):
                nc.vector.scalar_tensor_tensor(
                    out=scratch[:], in0=iota_sb, scalar=idxf[:, j:j + 1],
                    in1=orig[:], op0=Alu.is_equal, op1=Alu.mult,
                    accum_out=w8raw[:, j:j + 1],
                )
            sum8 = small.tile([128, 1], F32, tag="sum8")
            nc.vector.reduce_sum(sum8[:], w8raw[:], axis=X)
            rec = small.tile([128, 1], F32, tag="rec")
            nc.vector.reciprocal(rec[:], sum8[:])
            nc.vector.tensor_scalar(
                w_acc[:, t, :], w8raw[:], rec[:], ROUTE_SCALE,
                op0=Alu.mult, op1=Alu.mult,
            )

        nc.sync.dma_start(w_out_d[:], w_acc[:])
        nc.sync.dma_start(idx_out_d[:], idx_acc[:])

    nc.compile()
    return nc


def _prep_inputs(x, weight, bias):
    """Host-side sharding + layout transforms (all DMAs become contiguous)."""
    x = np.asarray(x, dtype=np.float32)
    weight = np.asarray(weight, dtype=np.float32)
    bias = np.asarray(bias, dtype=np.float32)

    # wt[p, k, e] = weight[e, k*128+p]
    wt = np.ascontiguousarray(
        weight.T.reshape(KD, 128, E).transpose(1, 0, 2)
    ).reshape(128, KD * E)
    bias_b = np.broadcast_to(bias, (128, E))
    iota = np.broadcast_to(np.arange(E, dtype=np.float32), (128, E))
    bi = np.ascontiguousarray(np.concatenate([bias_b, iota], axis=1))

    in_maps = []
    for c in range(NCORES):
        xs = x[c * TCORE:(c + 1) * TCORE]
        # xt[t, p, k*128+j] = xs[t*128+j, k*128+p]
        xt = np.ascontiguousarray(
            xs.reshape(NT, 128, KD, 128).transpose(0, 3, 2, 1)
        ).reshape(NT, 128, KD * 128)
        in_maps.append({"xt": xt, "wt": wt, "bi": bi})
    return in_maps


def _postprocess(results):
    ws, idxs = [], []
    for c in range(NCORES):
        w = results[c]["w_out"].reshape(128, NT, KTOP).transpose(1, 0, 2).reshape(TCORE, KTOP)
        ix = results[c]["idx_out"].reshape(128, NT, KTOP).transpose(1, 0, 2).reshape(TCORE, KTOP)
        ws.append(w)
        idxs.append(ix)
    w_full = np.concatenate(ws, axis=0).astype(np.float32)
    idx_full = np.concatenate(idxs, axis=0).astype(np.int32)
    return w_full, idx_full


def get_runner():
    """Build (once) and return a callable: in_maps -> per-core results list."""
    if "runner" in _CACHE:
        return _CACHE["runner"]

    from concourse.bass_utils import run_bass_kernel_spmd

    nc = _build()

    def runner(in_maps):
        return run_bass_kernel_spmd(nc, in_maps, list(range(NCORES))).results

    _CACHE["runner"] = runner
    _CACHE["nc"] = nc
    return runner


def kernel(x, weight, bias):
    runner = get_runner()
    in_maps = _prep_inputs(x, weight, bias)
    results = runner(in_maps)
    return _postprocess(results)


if __name__ == "__main__":
    rng = np.random.default_rng(0)
    x = rng.standard_normal((T, D), dtype=np.float32)
    w = rng.standard_normal((E, D), dtype=np.float32) * 0.02
    b = rng.standard_normal((E,), dtype=np.float32) * 0.02
    out_w, out_idx = kernel(x, w, b)
    print(out_w.shape, out_w.dtype, out_idx.shape, out_idx.dtype)
    print(out_w[0], out_idx[0])


# revision 2
# speedup vs baseline: 162.4686x; 162.4686x over previous
"""MoE gate routing kernel for Trainium2 (8 NeuronCores, SPMD token-parallel).

Problem: scores = sigmoid(x @ weight.T); s = scores + bias;
group top-2 sums -> top-4 groups mask -> global top-8 -> gather original
scores -> normalize * 2.5. Returns (w [T,8] f32, idx [T,8] int32).

Sharding: token dim split 8 ways; weight/bias replicated. Inside each core:
x^T tiles (prepared host-side) feed fp32 matmuls (exact top-k needs fp32);
routing block runs on DVE with max/max_index/match_replace ISA ops.
"""
import sys

if "/opt/trn_rl_repo" not in sys.path:
    sys.path.insert(0, "/opt/trn_rl_repo")

import numpy as np

T, D, E = 16384, 7168, 256
G, KG, KTOP = 8, 4, 8
ROUTE_SCALE = 2.5
NCORES = 8
TCORE = T // NCORES          # 2048 tokens per core
NT = TCORE // 128            # 16 token tiles per core
KD = D // 128                # 56 contraction chunks
BIG = 1e30

_CACHE = {}


def _build(bench_iters=0):
    import concourse.bacc as bacc
    import concourse.mybir as mybir
    import concourse.tile as tile
    from contextlib import ExitStack, nullcontext

    F32 = mybir.dt.float32
    U32 = mybir.dt.uint32
    X = mybir.AxisListType.X
    Alu = mybir.AluOpType

    nc = bacc.Bacc(None, target_bir_lowering=False, debug=False)

    xt_d = nc.dram_tensor("xt", [NT, 128, KD * 128], F32, kind="ExternalInput")
    wt_d = nc.dram_tensor("wt", [128, KD * E], F32, kind="ExternalInput")
    bi_d = nc.dram_tensor("bi", [128, 2 * E], F32, kind="ExternalInput")
    w_out_d = nc.dram_tensor("w_out", [128, NT * KTOP], F32, kind="ExternalOutput")
    idx_out_d = nc.dram_tensor("idx_out", [128, NT * KTOP], U32, kind="ExternalOutput")

    with tile.TileContext(nc) as tc, ExitStack() as ctx:
        const = ctx.enter_context(tc.tile_pool(name="const", bufs=1))
        outp = ctx.enter_context(tc.tile_pool(name="outp", bufs=1))
        xpool = ctx.enter_context(tc.tile_pool(name="xp", bufs=3))
        pspool = ctx.enter_context(tc.tile_pool(name="ps", bufs=2, space="PSUM"))
        work = ctx.enter_context(tc.tile_pool(name="work", bufs=2))
        small = ctx.enter_context(tc.tile_pool(name="small", bufs=2))

        wt_sb = const.tile([128, KD, E], F32)
        bi_sb = const.tile([128, 2 * E], F32)
        nc.sync.dma_start(wt_sb[:], wt_d[:])
        nc.sync.dma_start(bi_sb[:], bi_d[:])
        bias_sb = bi_sb[:, 0:E]
        iota_sb = bi_sb[:, E:2 * E]

        w_acc = outp.tile([128, NT, KTOP], F32)
        idx_acc = outp.tile([128, NT, KTOP], U32)

        loop_cm = tc.For_i(0, bench_iters, 1) if bench_iters else nullcontext()
        ctx.enter_context(loop_cm)
        for t in range(NT):
            xt = xpool.tile([128, KD, 128], F32, tag="xt")
            nc.sync.dma_start(xt[:], xt_d[t])

            ps = pspool.tile([128, E], F32, tag="ps")
            for k in range(KD):
                nc.tensor.matmul(
                    ps[:], xt[:, k, :], wt_sb[:, k, :],
                    start=(k == 0), stop=(k == KD - 1),
                )

            orig = work.tile([128, E], F32, tag="orig")
            nc.scalar.activation(orig[:], ps[:], mybir.ActivationFunctionType.Sigmoid)

            s = work.tile([128, E], F32, tag="s")
            nc.vector.tensor_add(s[:], orig[:], bias_sb)
            sg = s[:].rearrange("p (g f) -> p g f", g=G)

            m1 = small.tile([128, G], F32, tag="m1")
            nc.vector.reduce_max(m1[:], sg, axis=X)
            tmp = work.tile([128, E], F32, tag="tmp")
            nc.vector.match_replace(
                out=tmp[:], in_to_replace=m1[:], in_values=s[:], imm_value=-BIG
            )
            m2 = small.tile([128, G], F32, tag="m2")
            nc.vector.reduce_max(m2[:], tmp[:].rearrange("p (g f) -> p g f", g=G), axis=X)
            gs = small.tile([128, G], F32, tag="gs")
            nc.vector.tensor_add(gs[:], m1[:], m2[:])

            g8 = small.tile([128, 8], F32, tag="g8")
            nc.vector.max(out=g8[:], in_=gs[:])
            pen = small.tile([128, G], F32, tag="pen")
            nc.vector.tensor_scalar(
                pen[:], gs[:], g8[:, 3:4], -BIG, op0=Alu.is_lt, op1=Alu.mult
            )

            masked = work.tile([128, E], F32, tag="masked")
            pen_b = pen[:].unsqueeze(2).broadcast_to([128, G, E // G])
            nc.vector.tensor_tensor(
                out=masked[:].rearrange("p (g f) -> p g f", g=G),
                in0=sg, in1=pen_b, op=Alu.add,
            )

            v8 = small.tile([128, KTOP], F32, tag="v8")
            nc.vector.max(out=v8[:], in_=masked[:])
            nc.vector.max_index(idx_acc[:, t, :], v8[:], masked[:])

            idxf = small.tile([128, KTOP], F32, tag="idxf")
            nc.vector.tensor_copy(idxf[:], idx_acc[:, t, :])
            w8raw = small.tile([128, KTOP], F32, tag="w8raw")
            scratch = work.tile([128, E], F32, tag="scratch")
            for j in range(KTOP):
                nc.vector.scalar_tensor_tensor(
                    out=scratch[:], in0=iota_sb, scalar=idxf[:, j:j + 1],
                    in1=orig[:], op0=Alu.is_equal, op1=Alu.mult,
                    accum_out=w8raw[:, j:j + 1],
                )
            sum8 = small.tile([128, 1], F32, tag="sum8")
            nc.vector.reduce_sum(sum8[:], w8raw[:], axis=X)
            rec = small.tile([128, 1], F32, tag="rec")
            nc.vector.reciprocal(rec[:], sum8[:])
            nc.vector.tensor_scalar(
                w_acc[:, t, :], w8raw[:], rec[:], ROUTE_SCALE,
                op0=Alu.mult, op1=Alu.mult,
            )

        nc.sync.dma_start(w_out_d[:], w_acc[:])
        nc.sync.dma_start(idx_out_d[:], idx_acc[:])

    nc.compile()
    return nc


def _prep_inputs(x, weight, bias):
    """Host-side sharding + layout transforms (all DMAs become contiguous)."""
    x = np.asarray(x, dtype=np.float32)
    weight = np.asarray(weight, dtype=np.float32)
    bias = np.asarray(bias, dtype=np.float32)

    # wt[p, k, e] = weight[e, k*128+p]
    wt = np.ascontiguousarray(
        weight.T.reshape(KD, 128, E).transpose(1, 0, 2)
    ).reshape(128, KD * E)
    bias_b = np.broadcast_to(bias, (128, E))
    iota = np.broadcast_to(np.arange(E, dtype=np.float32), (128, E))
    bi = np.ascontiguousarray(np.concatenate([bias_b, iota], axis=1))

    in_maps = []
    for c in range(NCORES):
        xs = x[c * TCORE:(c + 1) * TCORE]
        # xt[t, p, k*128+j] = xs[t*128+j, k*128+p]
        xt = np.ascontiguousarray(
            xs.reshape(NT, 128, KD, 128).transpose(0, 3, 2, 1)
        ).reshape(NT, 128, KD * 128)
        in_maps.append({"xt": xt, "wt": wt, "bi": bi})
    return in_maps


def _postprocess(results):
    ws, idxs = [], []
    for c in range(NCORES):
        w = results[c]["w_out"].reshape(128, NT, KTOP).transpose(1, 0, 2).reshape(TCORE, KTOP)
        ix = results[c]["idx_out"].reshape(128, NT, KTOP).transpose(1, 0, 2).reshape(TCORE, KTOP)
        ws.append(w)
        idxs.append(ix)
    w_full = np.concatenate(ws, axis=0).astype(np.float32)
    idx_full = np.concatenate(idxs, axis=0).astype(np.int32)
    return w_full, idx_full


def get_runner():
    """Build (once) and return a callable: in_maps -> per-core results list."""
    if "runner" in _CACHE:
        return _CACHE["runner"]

    from concourse.bass_utils import run_bass_kernel_spmd

    nc = _build()

    def runner(in_maps):
        return run_bass_kernel_spmd(nc, in_maps, list(range(NCORES))).results

    _CACHE["runner"] = runner
    _CACHE["nc"] = nc
    return runner


def kernel(x, weight, bias):
    runner = get_runner()
    in_maps = _prep_inputs(x, weight, bias)
    results = runner(in_maps)
    return _postprocess(results)


if __name__ == "__main__":
    rng = np.random.default_rng(0)
    x = rng.standard_normal((T, D), dtype=np.float32)
    w = rng.standard_normal((E, D), dtype=np.float32) * 0.02
    b = rng.standard_normal((E,), dtype=np.float32) * 0.02
    out_w, out_idx = kernel(x, w, b)
    print(out_w.shape, out_w.dtype, out_idx.shape, out_idx.dtype)
    print(out_w[0], out_idx[0])
